# revision 13
# baseline (speedup 1.0000x reference)
"""Trainium2 Bass kernel for dense-MoE routing (8 experts, gate-weighted sum).

Math (restructured from the reference):
    gate   = softmax(x @ wg + bg)              per token, E=8
    h      = relu(x @ W1cat + b1cat)           W1cat = w1 transposed/concat [C, E*H]
    outT   = W2p.T @ (gate-scaled h).T + b2w.T @ gate.T
             W2p = w2.reshape(E*H,EO) @ wo (host-folded), b2w = b2@wo + bo
             (sum(gate)=1 absorbs bo);  out is computed transposed [OC, T]
             and the host transposes back.

All matmuls are bf16 with 512-row moving operands (full PE rate, dispatch
fully hidden).  mm2 keeps w2p stationary / hs moving so every PE matmul is
512 rows.  Stationary tiles serve both chunks of a 1024-token pair,
halving Ldweights.  Gate softmax: exp on ACT (bias fused), Z-sum via
gpsimd partition_all_reduce, no PE bias/sum matmuls.  Gate broadcast to
128 partitions via one DRAM-roundtrip broadcast DMA per chunk.  Output is
DMAed straight from PSUM.

Sharding: data-parallel over tokens; core i takes batch row i (4096 tokens).
"""

import numpy as np

_P = 128           # partitions
_T = 4096          # tokens per core
_TN = 512          # token chunk (psum bank width in f32)
_PAIR = 1024       # stationary-reuse unit (2 chunks)
_NPAIR = _T // _PAIR   # 4
_HM = 32           # hidden tiles (4096 / 128)
_E = 8
_OC = 256          # output channels
_NCORES = 8

_CACHE = {}


def _build_nc(reps=1, loop=1, stagger=False, p0split=True, obact=False,
              dummy_pe=0, dummy_act=0):
    import concourse.bacc as bacc
    import concourse.bass as bass
    import concourse.bass_isa as bass_isa
    import concourse.mybir as mybir
    import concourse.tile as tile

    f32 = mybir.dt.float32
    bf16 = mybir.dt.bfloat16
    AF = mybir.ActivationFunctionType
    ts = bass.ts

    nc = bacc.Bacc("TRN2", target_bir_lowering=False, debug=False)

    xT_d = nc.dram_tensor("xT", [_NPAIR, 2, _P, _PAIR], bf16, kind="ExternalInput").ap()
    w1_d = nc.dram_tensor("w1s", [2, 4, _P, 1024], bf16, kind="ExternalInput").ap()
    w2p_d = nc.dram_tensor("w2ps", [4, _P, 8, _OC], bf16, kind="ExternalInput").ap()
    b1_d = nc.dram_tensor("b1s", [_P, _HM], f32, kind="ExternalInput").ap()
    wg_d = nc.dram_tensor("wgs", [2, _P, _E], bf16, kind="ExternalInput").ap()
    bg_d = nc.dram_tensor("bgs", [_E, 1], f32, kind="ExternalInput").ap()
    b2w_d = nc.dram_tensor("b2ws", [_E, _OC], bf16, kind="ExternalInput").ap()
    gst_d = nc.dram_tensor("gstage", [2 * _NPAIR, 1, _E, _TN], bf16).ap()
    out_d = nc.dram_tensor("out", [2, _P, _T], bf16, kind="ExternalOutput").ap()

    NCH = 2 * _NPAIR  # 8 chunks

    with tile.TileContext(nc) as tc:
        with (
            tc.tile_pool(name="const", bufs=1) as const,
            tc.tile_pool(name="xt", bufs=2) as p_xt,
            tc.tile_pool(name="hs", bufs=2) as p_hs,
            tc.tile_pool(name="gb", bufs=3) as p_gb,
            tc.tile_pool(name="gsm", bufs=1) as p_gs,
            tc.tile_pool(name="gn", bufs=6) as p_gn,
            tc.tile_pool(name="ob", bufs=3) as p_ob,
            tc.tile_pool(name="psum_h", bufs=4, space="PSUM") as psum_h,
            tc.tile_pool(name="psum_o", bufs=3, space="PSUM") as psum_o,
            tc.tile_pool(name="psum_g", bufs=1, space="PSUM") as psum_g,
        ):
            w1_sb = const.tile([_P, 2, 4096], bf16, name="w1_sb")
            w2p_sb = const.tile([_P, _HM, _OC], bf16, name="w2p_sb")
            b1_sb = const.tile([_P, _HM], f32, name="b1_sb")
            wg_sb = const.tile([_P, 2, _E], bf16, name="wg_sb")
            bg_sb = const.tile([_E, 1], f32, name="bg_sb")
            b2w_sb = const.tile([_E, _OC], bf16, name="b2w_sb")

            xt_tiles = {}

            def load_xt(pr):
                xt = p_xt.tile([_P, 2, _PAIR], bf16, name="xt", tag="xt")
                for kc in range(2):
                    nc.sync.dma_start(out=xt[:, kc, :], in_=xT_d[pr, kc])
                xt_tiles[pr] = xt
                return xt

            # startup DMAs: gate weights + pair0 tokens first, then mm1
            # weights (quartered so hm=0 arrives early), then mm2 weights.
            for kc in range(2):
                nc.sync.dma_start(out=wg_sb[:, kc, :], in_=wg_d[kc])
            nc.sync.dma_start(out=bg_sb[:], in_=bg_d[:])
            load_xt(0)
            for q in range(4):
                for kc in range(2):
                    nc.sync.dma_start(out=w1_sb[:, kc, ts(q, 1024)], in_=w1_d[kc, q])
                if q == 0:
                    nc.sync.dma_start(out=b1_sb[:], in_=b1_d[:])
            load_xt(1)
            for q in range(4):
                nc.sync.dma_start(out=w2p_sb[:, ts(q, 8), :], in_=w2p_d[q])
            nc.sync.dma_start(out=b2w_sb[:], in_=b2w_d[:])
            for pr in range(2, _NPAIR):
                load_xt(pr)

            gatenb_tiles = {}
            gb_tiles = {}

            def emit_gate(ch):
                """Gate softmax for one 512-token chunk -> gatenb [E,TN] bf16
                (stationary for b2w matmul) + gb broadcast [P,E,TN]."""
                pr, c = divmod(ch, 2)
                xt = xt_tiles[pr]
                pg = psum_g.tile([_E, _TN], f32, name="pg", tag="pg")
                for kc in range(2):
                    nc.tensor.matmul(pg[:], wg_sb[:, kc, :],
                                     xt[:, kc, ts(c, _TN)],
                                     start=(kc == 0), stop=(kc == 1))
                expu = p_gs.tile([_E, _TN], f32, name="expu", tag="expu")
                nc.scalar.activation(expu[:], pg[:], AF.Exp, bias=bg_sb[:])
                z8 = p_gs.tile([_E, _TN], f32, name="z8", tag="z8")
                nc.gpsimd.partition_all_reduce(z8[:], expu[:], channels=_E,
                                               reduce_op=bass_isa.ReduceOp.add)
                rcb = p_gs.tile([_E, _TN], f32, name="rcb", tag="rcb")
                nc.vector.reciprocal(rcb[:], z8[:])
                gatenb = p_gn.tile([_E, _TN], bf16, name="gatenb", tag="gatenb")
                nc.vector.tensor_mul(gatenb[:], expu[:], rcb[:])
                nc.sync.dma_start(out=gst_d[ch, 0], in_=gatenb[:])
                gb = p_gb.tile([_P, _E, _TN], bf16, name="gb", tag="gb")
                src = gst_d[ch].broadcast_to((_P, _E, _TN))
                nc.sync.dma_start(out=gb[:], in_=src)
                gatenb_tiles[ch] = gatenb
                gb_tiles[ch] = gb

            def mm1_group(pr, hs, hm, split):
                """Per hidden tile: each stationary (hm,kc) serves both
                chunks of the pair; psum accumulates over kc."""
                xt = xt_tiles[pr]
                phs = [psum_h.tile([_P, _TN], f32, name="ph", tag="ph")
                       for _ in range(2)]
                for kc in range(2):
                    for c in range(2):
                        nc.tensor.matmul(phs[c][:], w1_sb[:, kc, ts(hm, _P)],
                                         xt[:, kc, ts(c, _TN)],
                                         start=(kc == 0), stop=(kc == 1))
                for c in range(2):
                    dst = hs[:, hm, ts(c, _TN)]
                    if split and c == 1:
                        nc.vector.tensor_scalar(dst, phs[c][:],
                                                b1_sb[:, hm:hm + 1], 0.0,
                                                mybir.AluOpType.add,
                                                mybir.AluOpType.max)
                    else:
                        nc.scalar.activation(dst, phs[c][:], AF.Relu,
                                             bias=b1_sb[:, hm:hm + 1])
                if hm % 4 == 3:
                    # gate-scale the finished expert block (4 hm tiles) in one
                    # DVE op per chunk: gb row repeated via zero-stride AP
                    e = hm // 4
                    for c in range(2):
                        gb = gb_tiles[2 * pr + c]
                        blk = hs[:, 4 * e:4 * e + 4, ts(c, _TN)]
                        rep = gb[:, e:e + 1, :].broadcast_to((_P, 4, _TN))
                        nc.vector.tensor_mul(blk, blk, rep)

            def finish_po(pr, oc, c, po):
                row = pr * _PAIR + c * _TN
                ob = p_ob.tile([_P, _TN], bf16, name="ob", tag="ob")
                if obact:
                    nc.scalar.copy(ob[:], po[:])
                else:
                    nc.vector.tensor_copy(ob[:], po[:])
                nc.sync.dma_start(out=out_d[oc, :, row:row + _TN], in_=ob[:])

            po_live = {}

            def mm2_steps(pr, hs):
                """Generator of mm2 PE groups for pair pr (oc-outer)."""
                for oc in range(2):
                    pos = [psum_o.tile([_P, _TN], f32, name="po", tag="po")
                           for _ in range(2)]
                    for kt in range(_HM):
                        for c in range(2):
                            nc.tensor.matmul(pos[c][:],
                                             w2p_sb[:, kt, ts(oc, _P)],
                                             hs[:, kt, ts(c, _TN)],
                                             start=(kt == 0), stop=False)
                        yield
                    for c in range(2):
                        nc.tensor.matmul(pos[c][:], b2w_sb[:, ts(oc, _P)],
                                         gatenb_tiles[2 * pr + c][:],
                                         start=False, stop=True)
                        finish_po(pr, oc, c, pos[c])
                    yield

            def emit_body():
                gatenb_tiles.clear()
                gb_tiles.clear()
                if loop > 1 or reps > 1:
                    # reload pair tokens each iteration so the loop body is
                    # self-contained (xt pool rotates; contents identical)
                    xt_tiles.clear()
                    for pr in range(_NPAIR):
                        load_xt(pr)
                emit_gate(0)
                emit_gate(1)
                pending = None
                for pr in range(_NPAIR):
                    hs = p_hs.tile([_P, _HM, _PAIR], bf16, name="hs", tag="hs")
                    for hm in range(_HM):
                        mm1_group(pr, hs, hm, split=(p0split and pr == 0))
                        if pending is not None:
                            next(pending, None)
                            next(pending, None)
                        if pr < _NPAIR - 1:
                            if hm == 4:
                                emit_gate(2 * pr + 2)
                            elif hm == 20:
                                emit_gate(2 * pr + 3)
                    if pending is not None:
                        for _ in pending:
                            pass
                    pending = mm2_steps(pr, hs)
                for _ in pending:
                    pass
                # timing-only perturbation probes (never used for output)
                for d in range(dummy_pe):
                    pd = psum_h.tile([_P, _TN], f32, name="pd", tag="ph")
                    nc.tensor.matmul(pd[:], w1_sb[:, 0, ts(d % 32, _P)],
                                     xt_tiles[0][:, 0, ts(0, _TN)],
                                     start=True, stop=True)
                for d in range(dummy_act):
                    da = p_ob.tile([_P, _TN], bf16, name="da", tag="ob")
                    nc.scalar.activation(da[:], hs[:, d % 32, ts(0, _TN)],
                                         AF.Relu)

            if loop > 1:
                with tc.For_i(0, loop, 1, staggered_reset=stagger):
                    for _r in range(reps):
                        emit_body()
            else:
                for _rep in range(reps):
                    emit_body()

    nc.compile()
    return nc


def _prep_weights(w1, b1, w2, b2, wg, bg, wo, bo):
    import ml_dtypes
    bf = ml_dtypes.bfloat16
    f32 = np.float32
    w1 = np.asarray(w1, f32)
    w2 = np.asarray(w2, f32)
    wo = np.asarray(wo, f32)
    E, IN, HID = w1.shape
    w1cat = w1.transpose(1, 0, 2).reshape(IN, E * HID)        # [256, 4096]
    w1s = np.ascontiguousarray(
        w1cat.reshape(2, _P, 4, 1024).transpose(0, 2, 1, 3)).astype(bf)
    w2p = (w2.astype(np.float64).reshape(E * HID, -1)
           @ wo.astype(np.float64)).astype(f32)               # [4096, 256]
    w2ps = np.ascontiguousarray(
        w2p.reshape(_HM, _P, _OC).transpose(1, 0, 2)
        .reshape(_P, 4, 8, _OC).transpose(1, 0, 2, 3)).astype(bf)
    b1s = np.ascontiguousarray(
        np.asarray(b1, f32).reshape(E * HID).reshape(_HM, _P).T)
    b2ws = (np.asarray(b2, np.float64) @ wo.astype(np.float64)
            + np.asarray(bo, np.float64)).astype(f32).astype(bf)
    wgs = np.ascontiguousarray(np.asarray(wg, f32).reshape(2, _P, E)).astype(bf)
    bgs = np.ascontiguousarray(np.asarray(bg, f32).reshape(E, 1))
    return dict(w1s=w1s, w2ps=w2ps, b1s=b1s, b2ws=b2ws, wgs=wgs, bgs=bgs)


def _prep_x(x2d, core):
    import ml_dtypes
    xc = x2d[core * _T:(core + 1) * _T]                        # [T, C]
    return np.ascontiguousarray(
        xc.T.reshape(2, _P, _NPAIR, _PAIR).transpose(2, 0, 1, 3)
    ).astype(ml_dtypes.bfloat16)


def _run(x, w1, b1, w2, b2, wg, bg, wo, bo, trace=False):
    from concourse.bass_utils import run_bass_kernel_spmd

    if "nc" not in _CACHE:
        _CACHE["nc"] = _build_nc(1)
    nc = _CACHE["nc"]

    x = np.asarray(x, np.float32)
    b, n, c = x.shape
    weights = _prep_weights(w1, b1, w2, b2, wg, bg, wo, bo)
    x2d = x.reshape(b * n, c)
    in_maps = [{"xT": _prep_x(x2d, i), **weights} for i in range(_NCORES)]

    res = run_bass_kernel_spmd(nc, in_maps, list(range(_NCORES)), trace=trace)
    outs = []
    for i in range(_NCORES):
        o = np.asarray(res.results[i]["out"], np.float32)      # [2, 128, T]
        outs.append(o.reshape(_OC, _T).T)                      # [T, OC]
    out = np.concatenate(outs, axis=0)
    return out.reshape(b, n, _OC), res


def kernel(x, w1, b1, w2, b2, wg, bg, wo, bo):
    out, _ = _run(x, w1, b1, w2, b2, wg, bg, wo, bo, trace=False)
    return out
